# revision 14
# baseline (speedup 1.0000x reference)
"""DFlash draft-model (block-sparse attention + CE loss) Trainium2 kernel.

Sharding: 8 cores = 2 batches x 4 block-quarters. Each core computes its
batch's context K/V (replicated within the 4-core group), attention for its
32 anchor blocks (512 draft tokens), and the full-vocab CE partials
(sum-exp + target logit) for its tokens. Host combines 8x512 scalars.
"""
import sys

if '/opt/trn_rl_repo' not in sys.path:
    sys.path.insert(0, '/opt/trn_rl_repo')

import numpy as np
import ml_dtypes

import concourse.bass as bass
import concourse.tile as tile
from concourse import mybir
from concourse.bass_utils import run_bass_kernel_spmd
from concourse.vector_clock import ScopedClock
from concourse.masks import make_identity

BF16 = ml_dtypes.bfloat16
F32 = mybir.dt.float32
BF = mybir.dt.bfloat16
F8 = mybir.dt.float8e4
AF = mybir.ActivationFunctionType
ALU = mybir.AluOpType

B, S, D, H, V = 2, 2048, 1024, 8, 32000
N_ANC, BS = 128, 16
HD = D // H            # 128
NCORES = 8
NB = N_ANC // 4        # 32 blocks per core
T = NB * BS            # 512 tokens per core
KW = S + 2 * HD        # hmm: draft cols = 128 -> total score width
KWID = S + 128         # 2176 score columns (2048 ctx + 128 group draft)
GAMMA, EPS = 7.0, 1e-6
MASK_ID = V - 1
NEG = -30000.0
VCH = [1024] * 31 + [256]        # vocab chunks (sum = 32000)
F8SCALE = 16.0                   # lm_head fp8 pre-scale

# ---------------------------------------------------------------------------
# Workaround: this container's walrus rejects >1 sync-wait per instruction on
# the Tile kernel-tail drain; split the waits across several SP drains.
_MAX_WAITS = 1


def _patched_drain_and_barrier(self, tick_clock, wait_clock):
    nc = self.nc
    drain_inst = nc.sync.drain()
    wait_clock.add_sem_waits(
        drain_inst.ins, ScopedClock({None: tick_clock.global_clock})
    )
    si = drain_inst.ins.sync_info
    waits = list(si.on_wait)
    if len(waits) > _MAX_WAITS:
        si.on_wait = waits[:_MAX_WAITS]
        rest = waits[_MAX_WAITS:]
        for i in range(0, len(rest), _MAX_WAITS):
            extra = nc.sync.drain()
            extra.ins.sync_info = mybir.SyncInfo(
                on_update=[], on_wait=rest[i:i + _MAX_WAITS]
            )
    nc.all_engine_barrier()
    assert self.sems is not None
    popped = nc._tile_sem_poison_stack.pop()
    assert popped is self._sem_poison
    nc.clear_and_free_semaphores(list(self.sems.allocated().values()))
    nc.all_engine_barrier()


tile.TileContext._drain_and_barrier = _patched_drain_and_barrier


def _split_waits(nc, max_waits=_MAX_WAITS):
    """Walrus here allows only one sync-wait per instruction; hoist extra
    waits onto same-engine NOPs inserted immediately before the instruction
    (same engine stream order => identical semantics)."""
    for fn in nc.m.functions:
        for bb in fn.blocks:
            out = []
            changed = False
            for inst in bb.instructions:
                si = getattr(inst, "sync_info", None)
                waits = list(si.on_wait) if si is not None and si.on_wait else []
                if len(waits) > max_waits:
                    changed = True
                    keep = waits[-max_waits:]
                    rest = waits[:-max_waits]
                    for i in range(0, len(rest), max_waits):
                        nop = mybir.InstEventSemaphore(
                            name=nc.get_next_instruction_name(),
                            ins=[], outs=[])
                        nop.engine = inst.engine
                        nop.sync_info = mybir.SyncInfo(
                            on_update=[], on_wait=rest[i:i + max_waits])
                        out.append(nop)
                    si.on_wait = keep
                out.append(inst)
            if changed:
                bb.instructions = out
# ---------------------------------------------------------------------------


def _rope6(nc, pool, psrc, dst, cos_t, sin_t):
    """RoPE in [token-partition, D-free] layout.

    psrc: [128, 1024] fp32 (PSUM), viewed [128, H, 2, 64].
    dst:  [128, H, 2, 64] bf16 SBUF tile.
    cos_t/sin_t: [128, 64] fp32 (same angles for every head).
    """
    pv = psrc.rearrange("p (h two c) -> p h two c", h=H, two=2)
    x1, x2 = pv[:, :, 0, :], pv[:, :, 1, :]
    cosb = cos_t[:, None, :].to_broadcast((128, H, 64))
    sinb = sin_t[:, None, :].to_broadcast((128, H, 64))
    t1 = pool.tile([128, H, 64], F32, tag="rope_t1")
    t2 = pool.tile([128, H, 64], F32, tag="rope_t2")
    nc.vector.tensor_tensor(t1[:], x1, cosb, ALU.mult)
    nc.vector.tensor_tensor(t2[:], x2, sinb, ALU.mult)
    nc.vector.tensor_tensor(dst[:, :, 0, :], t1[:], t2[:], ALU.subtract)
    nc.vector.tensor_tensor(t1[:], x1, sinb, ALU.mult)
    nc.vector.tensor_tensor(t2[:], x2, cosb, ALU.mult)
    nc.vector.tensor_tensor(dst[:, :, 1, :], t1[:], t2[:], ALU.add)


def _build_nc():
    nc = bass.Bass("TRN2", target_bir_lowering=False, debug=False,
                   num_devices=NCORES)
    d = {}
    def di(name, shape, dt):
        d[name] = nc.dram_tensor(name, shape, dt, kind="ExternalInput").ap()
    di("hsT", [3 * D, S], BF)
    di("wctx", [3 * D, D], BF)
    di("wq", [128, 8, D], BF)  # pre-tiled [p, kc, n]; pre-scaled by 1/sqrt(HD)
    di("wk", [128, 8, D], BF)
    di("wv", [128, 8, D], BF)
    di("wo", [128, 8, D], BF)
    di("noiseT", [128, 8, T], BF)
    di("cosd", [T, 64], F32)
    di("sind", [T, 64], F32)
    di("cosc", [S, 64], F32)
    di("sinc", [S, 64], F32)
    di("mask", [T, KWID], F32)
    di("lmtgt", [T, D], BF)
    di("lmTa", [31, 128, 8 * 1024], F8)
    di("lmTb", [128, 8 * 256], F8)
    di("normw", [1, D], F32)
    se = nc.dram_tensor("se", [4, 128], F32, kind="ExternalOutput").ap()
    tl = nc.dram_tensor("tl", [4, 128], F32, kind="ExternalOutput").ap()

    with tile.TileContext(nc) as tc:
        _body(nc, tc, d, se, tl)
    _split_waits(nc)
    return nc


def _body(nc, tc, d, se_out, tl_out):
    from contextlib import ExitStack
    ctx = ExitStack()
    with ctx:
        pmain = ctx.enter_context(tc.tile_pool(name="pmain", bufs=1))
        pstr = ctx.enter_context(tc.tile_pool(name="pstream", bufs=3))
        psmall = ctx.enter_context(tc.tile_pool(name="psmall", bufs=2))

        ident = pmain.tile([128, 128], BF)
        make_identity(nc, ident[:])
        normw = pmain.tile([128, D], F32)
        nc.sync.dma_start(normw[:], d["normw"].to_broadcast((128, D)))
        eps_t = pmain.tile([128, 1], F32)
        nc.vector.memset(eps_t[:], EPS)

        oT = pmain.tile([128, H, T], BF)        # written D, read E
        hidT = pmain.tile([128, H, T], F8)      # written E, read F

        with tc.tile_pool(name="pkv", bufs=1) as pkv:
            kT = pkv.tile([128, H, S], BF)      # [hd, head, s]
            vv = pkv.tile([128, 16, D], BF)     # [s-in-tile, stile, col]

            # ------------ Stage A: ctxT = (W_ctx^T @ hs_cat^T) ------------
            with tc.tile_pool(name="pctxT", bufs=1) as pctxT:
                ctxT = pctxT.tile([128, 8, S], BF)  # [dchunk-row, dtile, s]
                with tc.tile_pool(name="psA", bufs=1, space="PSUM") as psA:
                    for sc in range(4):
                        pa = [psA.tile([128, 512], F32, tag=f"ctxps{dt}",
                                       name=f"ctxps{dt}")
                              for dt in range(8)]
                        for kc in range(24):
                            hst = pstr.tile([128, 512], BF, tag="hst")
                            nc.sync.dma_start(
                                hst[:],
                                d["hsT"][kc * 128:(kc + 1) * 128,
                                         sc * 512:(sc + 1) * 512])
                            wcs = pstr.tile([128, D], BF, tag="wcs")
                            nc.sync.dma_start(
                                wcs[:], d["wctx"][kc * 128:(kc + 1) * 128, :])
                            for dt in range(8):
                                nc.tensor.matmul(
                                    pa[dt][:],
                                    lhsT=wcs[:, dt * 128:(dt + 1) * 128],
                                    rhs=hst[:],
                                    start=(kc == 0), stop=(kc == 23))
                        for dt in range(8):
                            nc.any.tensor_copy(
                                ctxT[:, dt, sc * 512:(sc + 1) * 512], pa[dt][:])

                # ------------ Stage B: k_ctx (rope) / v_ctx ---------------
                with tc.tile_pool(name="pwB", bufs=1) as pwB, \
                     tc.tile_pool(name="prope", bufs=2) as prope, \
                     tc.tile_pool(name="psB", bufs=2, space="PSUM") as psB, \
                     tc.tile_pool(name="psBt", bufs=2, space="PSUM") as psBt:
                    wk_sb = pwB.tile([128, 8, D], BF)
                    nc.sync.dma_start(wk_sb[:], d["wk"][:])
                    wv_sb = pwB.tile([128, 8, D], BF)
                    nc.sync.dma_start(wv_sb[:], d["wv"][:])
                    for st in range(16):
                        pk = psB.tile([128, D], F32, tag="bkv")
                        for half in range(2):
                            for kc in range(8):
                                nc.tensor.matmul(
                                    pk[:, half * 512:(half + 1) * 512],
                                    lhsT=ctxT[:, kc, st * 128:(st + 1) * 128],
                                    rhs=wk_sb[:, kc,
                                              half * 512:(half + 1) * 512],
                                    start=(kc == 0), stop=(kc == 7))
                        cos_t = prope.tile([128, 64], F32, tag="cosc")
                        nc.sync.dma_start(
                            cos_t[:], d["cosc"][st * 128:(st + 1) * 128, :])
                        sin_t = prope.tile([128, 64], F32, tag="sinc")
                        nc.sync.dma_start(
                            sin_t[:], d["sinc"][st * 128:(st + 1) * 128, :])
                        krot = prope.tile([128, H, 2, 64], BF, tag="krot")
                        _rope6(nc, prope, pk[:], krot, cos_t, sin_t)
                        for h in range(H):
                            pt = psBt.tile([128, 128], BF, tag="btp")
                            nc.tensor.transpose(
                                pt[:],
                                krot[:, h].rearrange("p a b -> p (a b)"),
                                ident[:])
                            nc.any.tensor_copy(
                                kT[:, h, st * 128:(st + 1) * 128], pt[:])
                        pv = psB.tile([128, D], F32, tag="bkv")
                        for half in range(2):
                            for kc in range(8):
                                nc.tensor.matmul(
                                    pv[:, half * 512:(half + 1) * 512],
                                    lhsT=ctxT[:, kc, st * 128:(st + 1) * 128],
                                    rhs=wv_sb[:, kc,
                                              half * 512:(half + 1) * 512],
                                    start=(kc == 0), stop=(kc == 7))
                        nc.any.tensor_copy(vv[:, st, :], pv[:])

            # ---------------- Stages C + D ------------------------------
            with tc.tile_pool(name="pdraft", bufs=1) as pdraft:
                qT = pdraft.tile([128, H, T], BF)
                kdT = pdraft.tile([128, H, T], BF)
                vd = pdraft.tile([128, 4, D], BF)

                # ------------ Stage C: draft q / k / v --------------------
                with tc.tile_pool(name="pwC", bufs=1) as pwC, \
                     tc.tile_pool(name="prope2", bufs=2) as prope2, \
                     tc.tile_pool(name="psC", bufs=2, space="PSUM") as psC, \
                     tc.tile_pool(name="psCt", bufs=2, space="PSUM") as psCt:
                    wq_sb = pwC.tile([128, 8, D], BF)
                    nc.sync.dma_start(wq_sb[:], d["wq"][:])
                    wk2_sb = pwC.tile([128, 8, D], BF)
                    nc.sync.dma_start(wk2_sb[:], d["wk"][:])
                    wv2_sb = pwC.tile([128, 8, D], BF)
                    nc.sync.dma_start(wv2_sb[:], d["wv"][:])
                    noiseT_sb = pwC.tile([128, 8, T], BF)
                    nc.sync.dma_start(noiseT_sb[:], d["noiseT"][:])
                    for tt in range(4):
                        cos_t = prope2.tile([128, 64], F32, tag="cosd")
                        nc.sync.dma_start(
                            cos_t[:], d["cosd"][tt * 128:(tt + 1) * 128, :])
                        sin_t = prope2.tile([128, 64], F32, tag="sind")
                        nc.sync.dma_start(
                            sin_t[:], d["sind"][tt * 128:(tt + 1) * 128, :])
                        for wsb, dstT in ((wq_sb, qT), (wk2_sb, kdT)):
                            pq = psC.tile([128, D], F32, tag="ckv")
                            for half in range(2):
                                for kc in range(8):
                                    nc.tensor.matmul(
                                        pq[:, half * 512:(half + 1) * 512],
                                        lhsT=noiseT_sb[:, kc,
                                                       tt * 128:(tt + 1) * 128],
                                        rhs=wsb[:, kc,
                                                half * 512:(half + 1) * 512],
                                        start=(kc == 0), stop=(kc == 7))
                            rot = prope2.tile([128, H, 2, 64], BF, tag="drot")
                            _rope6(nc, prope2, pq[:], rot, cos_t, sin_t)
                            for h in range(H):
                                pt = psCt.tile([128, 128], BF, tag="ctp")
                                nc.tensor.transpose(
                                    pt[:],
                                    rot[:, h].rearrange("p a b -> p (a b)"),
                                    ident[:])
                                nc.any.tensor_copy(
                                    dstT[:, h, tt * 128:(tt + 1) * 128], pt[:])
                        pq = psC.tile([128, D], F32, tag="ckv")
                        for half in range(2):
                            for kc in range(8):
                                nc.tensor.matmul(
                                    pq[:, half * 512:(half + 1) * 512],
                                    lhsT=noiseT_sb[:, kc,
                                                   tt * 128:(tt + 1) * 128],
                                    rhs=wv2_sb[:, kc,
                                               half * 512:(half + 1) * 512],
                                    start=(kc == 0), stop=(kc == 7))
                        nc.any.tensor_copy(vd[:, tt, :], pq[:])

                # ------------ Stage D: block-sparse attention -------------
                with tc.tile_pool(name="pattn", bufs=3) as pattn, \
                     tc.tile_pool(name="psDs", bufs=2, space="PSUM") as psDs, \
                     tc.tile_pool(name="psDd", bufs=2, space="PSUM") as psDd, \
                     tc.tile_pool(name="psDt", bufs=2, space="PSUM") as psDt, \
                     tc.tile_pool(name="psDo", bufs=2, space="PSUM") as psDo:
                    for grp in range(4):
                        mask_t = pattn.tile([128, KWID], F32, tag="mask")
                        nc.sync.dma_start(
                            mask_t[:], d["mask"][grp * 128:(grp + 1) * 128, :])
                        for h in range(H):
                            s_sb = pattn.tile([128, KWID], F32, tag="scores")
                            for ck in range(4):
                                ps = psDs.tile([128, 512], F32, tag="sc")
                                nc.tensor.matmul(
                                    ps[:],
                                    lhsT=qT[:, h, grp * 128:(grp + 1) * 128],
                                    rhs=kT[:, h, ck * 512:(ck + 1) * 512],
                                    start=True, stop=True)
                                nc.vector.tensor_tensor(
                                    s_sb[:, ck * 512:(ck + 1) * 512], ps[:],
                                    mask_t[:, ck * 512:(ck + 1) * 512],
                                    ALU.add)
                            psd = psDd.tile([128, 128], F32, tag="sd")
                            nc.tensor.matmul(
                                psd[:],
                                lhsT=qT[:, h, grp * 128:(grp + 1) * 128],
                                rhs=kdT[:, h, grp * 128:(grp + 1) * 128],
                                start=True, stop=True)
                            nc.vector.tensor_tensor(
                                s_sb[:, S:], psd[:], mask_t[:, S:], ALU.add)
                            l_t = psmall.tile([128, 1], F32, tag="lsum")
                            nc.scalar.activation(out=s_sb[:], in_=s_sb[:],
                                                 func=AF.Exp,
                                                 accum_out=l_t[:])
                            rl = psmall.tile([128, 1], F32, tag="rl")
                            nc.vector.reciprocal(rl[:], l_t[:])
                            p_bf = pattn.tile([128, 17, 128], BF, tag="pbf")
                            nc.scalar.activation(
                                out=p_bf[:].rearrange("p a b -> p (a b)"),
                                in_=s_sb[:], func=AF.Copy, scale=rl[:])
                            pT_sb = pattn.tile([128, 17, 128], BF, tag="pT")
                            for ck in range(17):
                                pt = psDt.tile([128, 128], BF, tag="dtp")
                                nc.tensor.transpose(pt[:], p_bf[:, ck, :],
                                                    ident[:])
                                nc.any.tensor_copy(pT_sb[:, ck, :], pt[:])
                            po = psDo.tile([128, 128], F32, tag="po")
                            for ck in range(16):
                                nc.tensor.matmul(
                                    po[:],
                                    lhsT=vv[:, ck, h * 128:(h + 1) * 128],
                                    rhs=pT_sb[:, ck, :],
                                    start=(ck == 0), stop=False)
                            nc.tensor.matmul(
                                po[:],
                                lhsT=vd[:, grp, h * 128:(h + 1) * 128],
                                rhs=pT_sb[:, 16, :],
                                start=False, stop=True)
                            nc.any.tensor_copy(
                                oT[:, h, grp * 128:(grp + 1) * 128], po[:])

        # ---------------- Stage E: Wo + RMSNorm + target logit ------------
        with tc.tile_pool(name="pE", bufs=2) as pE, \
             tc.tile_pool(name="pwE", bufs=1) as pwE, \
             tc.tile_pool(name="psE", bufs=2, space="PSUM") as psE, \
             tc.tile_pool(name="psEt", bufs=2, space="PSUM") as psEt:
            wo_sb = pwE.tile([128, 8, D], BF)
            nc.sync.dma_start(wo_sb[:], d["wo"][:])
            for tt in range(4):
                ph = psE.tile([128, D], F32, tag="hid")
                for half in range(2):
                    for kc in range(8):
                        nc.tensor.matmul(
                            ph[:, half * 512:(half + 1) * 512],
                            lhsT=oT[:, kc, tt * 128:(tt + 1) * 128],
                            rhs=wo_sb[:, kc, half * 512:(half + 1) * 512],
                            start=(kc == 0), stop=(kc == 7))
                sq = pE.tile([128, D], F32, tag="sq")
                ssq = psmall.tile([128, 1], F32, tag="ssq")
                nc.scalar.activation(out=sq[:], in_=ph[:], func=AF.Square,
                                     accum_out=ssq[:])
                rms = psmall.tile([128, 1], F32, tag="rms")
                nc.scalar.activation(out=rms[:], in_=ssq[:], func=AF.Sqrt,
                                     bias=eps_t[:], scale=1.0 / D)
                rinv = psmall.tile([128, 1], F32, tag="rinv")
                nc.vector.reciprocal(rinv[:], rms[:])
                hid_f = pE.tile([128, D], F32, tag="hidf")
                nc.scalar.activation(out=hid_f[:], in_=ph[:], func=AF.Copy,
                                     scale=rinv[:])
                hid_b = pE.tile([128, D], BF, tag="hidb")
                nc.vector.tensor_tensor(hid_b[:], hid_f[:], normw[:], ALU.mult)
                lmt = pE.tile([128, D], BF, tag="lmt")
                nc.sync.dma_start(lmt[:],
                                  d["lmtgt"][tt * 128:(tt + 1) * 128, :])
                tl_t = psmall.tile([128, 1], F32, tag="tlt")
                prod = pE.tile([128, D], F32, tag="tprod")
                nc.vector.tensor_tensor(prod[:], hid_b[:], lmt[:], ALU.mult)
                nc.vector.reduce_sum(tl_t[:], prod[:],
                                     axis=mybir.AxisListType.X)
                nc.sync.dma_start(tl_out[tt, :], tl_t[:, 0])
                for kc in range(8):
                    pt = psEt.tile([128, 128], BF, tag="etp")
                    nc.tensor.transpose(
                        pt[:], hid_b[:, kc * 128:(kc + 1) * 128], ident[:])
                    nc.any.tensor_copy(hidT[:, kc, tt * 128:(tt + 1) * 128],
                                       pt[:])

        # ---------------- Stage F: lm_head + sum-exp ----------------------
        with tc.tile_pool(name="pF", bufs=3) as pF, \
             tc.tile_pool(name="pFs", bufs=3) as pFs, \
             tc.tile_pool(name="pFse", bufs=1) as pFse, \
             tc.tile_pool(name="psF", bufs=3, space="PSUM") as psF:
            sech = [pFse.tile([128, len(VCH)], F32, tag=f"sech{tt}",
                              name=f"sech{tt}")
                    for tt in range(4)]
            DR = mybir.MatmulPerfMode.DoubleRow
            off = 0
            for vc, vcw in enumerate(VCH):
                lmw = pF.tile([128, 8, 1024], F8, tag="lmw")
                if vcw == 1024:
                    nc.sync.dma_start(
                        lmw[:],
                        d["lmTa"][vc].rearrange("p (a b) -> p a b", a=8))
                else:
                    nc.sync.dma_start(
                        lmw[:, :, :vcw],
                        d["lmTb"][:].rearrange("p (a b) -> p a b", a=8))
                for tt in range(4):
                    ps = psF.tile([128, 1024], F32, tag="lg")
                    for hf in range(0, vcw, 512):
                        hw_ = min(512, vcw - hf)
                        for k2 in range(4):
                            nc.tensor.matmul(
                                ps[:, hf:hf + hw_],
                                lhsT=hidT[:, 2 * k2:2 * k2 + 2,
                                          tt * 128:(tt + 1) * 128],
                                rhs=lmw[:, 2 * k2:2 * k2 + 2, hf:hf + hw_],
                                start=(k2 == 0), stop=(k2 == 3),
                                perf_mode=DR)
                    scr = pFs.tile([128, 1024], F32, tag="escr")
                    nc.scalar.activation(out=scr[:, :vcw], in_=ps[:, :vcw],
                                         func=AF.Exp, scale=1.0 / F8SCALE,
                                         accum_out=sech[tt][:, vc:vc + 1])
                off += vcw
            for tt in range(4):
                se_t = psmall.tile([128, 1], F32, tag="set")
                nc.vector.reduce_sum(se_t[:], sech[tt][:],
                                     axis=mybir.AxisListType.X)
                nc.sync.dma_start(se_out[tt, :], se_t[:, 0])


_NC_CACHE = None


def _get_nc():
    global _NC_CACHE
    if _NC_CACHE is None:
        _NC_CACHE = _build_nc()
    return _NC_CACHE


def _prep_core_inputs(inputs):
    ids = np.asarray(inputs["input_ids"])
    hs0 = np.asarray(inputs["hs0"], dtype=np.float32)
    hs1 = np.asarray(inputs["hs1"], dtype=np.float32)
    hs2 = np.asarray(inputs["hs2"], dtype=np.float32)
    loss_mask = np.asarray(inputs["loss_mask"], dtype=np.float32)
    lm_head = np.asarray(inputs["lm_head_weight"], dtype=np.float32)
    anchors = np.asarray(inputs["anchor_positions"]).astype(np.int64)
    keep = np.asarray(inputs["block_keep_mask"]).astype(bool)
    embed = np.asarray(inputs["embed"], dtype=np.float32)
    w_ctx = np.asarray(inputs["W_ctx"], dtype=np.float32)
    wq = np.asarray(inputs["Wq"], dtype=np.float32)
    wk = np.asarray(inputs["Wk"], dtype=np.float32)
    wv = np.asarray(inputs["Wv"], dtype=np.float32)
    wo = np.asarray(inputs["Wo"], dtype=np.float32)
    norm_w = np.asarray(inputs["norm_weight"], dtype=np.float32)

    inv = (1.0 / (10000.0 ** (np.arange(64, dtype=np.float32) / np.float32(64)))
           ).astype(np.float32)
    ang_c = np.arange(S, dtype=np.float32)[:, None] * inv[None, :]
    cosc = np.cos(ang_c).astype(np.float32)
    sinc = np.sin(ang_c).astype(np.float32)
    offs = np.arange(BS)
    decay = np.exp(-np.clip(offs - 1, 0, None).astype(np.float32) / GAMMA)

    def tile_w(w):
        # [D, D] -> [p, kc, n]: row (kc*128+p) -> [p, kc, :]
        return np.ascontiguousarray(
            w.reshape(8, 128, D).transpose(1, 0, 2)).astype(BF16)

    lm8 = (np.ascontiguousarray(lm_head.T) * np.float32(F8SCALE)).astype(
        ml_dtypes.float8_e4m3)                      # [D, V]
    lm8_t = np.ascontiguousarray(
        lm8.reshape(8, 128, V).transpose(1, 0, 2))  # [128, 8, V]
    lmTa = np.ascontiguousarray(
        lm8_t[:, :, :31 * 1024].reshape(128, 8, 31, 1024)
        .transpose(2, 0, 1, 3)).reshape(31, 128, 8 * 1024)
    lmTb = np.ascontiguousarray(
        lm8_t[:, :, 31 * 1024:]).reshape(128, 8 * 256)
    common = {
        "wctx": w_ctx.astype(BF16),
        "wq": tile_w(wq / np.sqrt(np.float32(HD))),
        "wk": tile_w(wk),
        "wv": tile_w(wv),
        "wo": tile_w(wo),
        "lmTa": lmTa,
        "lmTb": lmTb,
        "normw": norm_w.reshape(1, D).astype(np.float32),
        "cosc": cosc, "sinc": sinc,
    }
    hsT_by_batch = [
        np.ascontiguousarray(
            np.concatenate([hs0[b], hs1[b], hs2[b]], axis=-1).T).astype(BF16)
        for b in range(B)
    ]
    e_mask = embed[MASK_ID]

    in_maps, host_w = [], []
    for c in range(NCORES):
        b, q4 = divmod(c, 4)
        nsl = slice(q4 * NB, (q4 + 1) * NB)
        anc = anchors[b, nsl]                     # [32]
        kp = keep[b, nsl]                         # [32]
        safe_anc = np.clip(anc, 0, S - 1)
        start_tok = np.where(kp, ids[b, safe_anc], MASK_ID)

        noise = np.broadcast_to(e_mask, (NB, BS, D)).copy()
        noise[:, 0, :] = embed[start_tok]
        noiseT = np.ascontiguousarray(
            noise.reshape(T, D).T.reshape(8, 128, T).transpose(1, 0, 2)
        ).astype(BF16)

        pos = (anc[:, None] + offs[None, :]).reshape(T)     # [512]
        ang_d = pos.astype(np.float32)[:, None] * inv[None, :]
        cosd = np.cos(ang_d).astype(np.float32)
        sind = np.sin(ang_d).astype(np.float32)

        m = np.full((T, KWID), NEG, dtype=np.float32)
        m_ctx = np.where(
            (np.arange(S)[None, :] < anc[:, None]) & kp[:, None],
            np.float32(0), np.float32(NEG))               # [32, 2048]
        m[:, :S] = np.repeat(m_ctx, BS, axis=0)
        blk_eye = np.kron(np.eye(8, dtype=bool), np.ones((BS, BS), bool))
        for g in range(4):
            kp_g = np.repeat(kp[g * 8:(g + 1) * 8], BS)   # [128]
            allow = blk_eye & kp_g[None, :] & kp_g[:, None]
            m[g * 128:(g + 1) * 128, S:] = np.where(allow, 0.0, NEG)

        tpos = np.clip(pos, 0, S - 1)
        tgt = ids[b, tpos]
        lmtgt = lm_head[tgt].astype(BF16)

        valid = pos < S
        j_gt0 = np.tile(offs > 0, NB)
        w = (np.repeat(kp, BS) & valid).astype(np.float32)
        w = w * j_gt0.astype(np.float32) * loss_mask[b, tpos]
        w = w * np.tile(decay, NB)
        host_w.append(w)

        im = dict(common)
        im["hsT"] = hsT_by_batch[b]
        im["noiseT"] = noiseT
        im["cosd"] = cosd
        im["sind"] = sind
        im["mask"] = m
        im["lmtgt"] = lmtgt
        in_maps.append(im)
    return in_maps, host_w


def _combine(results, host_w):
    num = np.float64(0.0)
    den = np.float64(0.0)
    for c in range(NCORES):
        se = np.asarray(results[c]["se"], np.float64).reshape(T)
        tl = np.asarray(results[c]["tl"], np.float64).reshape(T)
        w = host_w[c].astype(np.float64)
        lpt = np.log(np.maximum(se, 1e-300)) - tl
        num += np.sum(np.where(w > 0, lpt, 0.0) * w)
        den += np.sum(w)
    return np.float32(num / max(den, 1.0))


def kernel(**inputs):
    in_maps, host_w = _prep_core_inputs(inputs)
    nc = _get_nc()
    res = run_bass_kernel_spmd(nc, in_maps, core_ids=list(range(NCORES)))
    return _combine(res.results, host_w)
